# revision 52
# baseline (speedup 1.0000x reference)
"""Trainium2 Bass kernel for nn_EqStftPBC (STFT perturbation-based compensation).

Per (batch b, mode m):
  X = STFT(x); C_n2 = X*conj(roll(X,n2)); U_n2 = circ(w[:,n2]) @ C (+ time-roll);
  V_n2 = U_n2 * roll(X,n2); delta_f = sum_n2 V_n2; y = ISTFT(delta)*P
8 cores = (b x m x n2-half); per-core data-only variation (S/M stacks).

v5 (~32us vs 52us baseline):
- ALL big elementwise ops on vector only: a co-running gpsimd halves DVE
  throughput (measured), so gpsimd only issues DMAs/memsets.
- j-sum folded into PSUM via zero-stride matmul dst (one [80,102] accumulator
  for all 40 G-matmul contributions); only ONE accumulation window may be
  open per PSUM bank, so a zero-matmul opens the region.
- ViN never materialized: gpass1 splits into GiN=-Gi and Gi half-matmuls.
- time-roll U += roll_t(U) folded into the post-eviction step as one flat TT
  per psum bank via ghost slots (ghosts filled by tiny gpsimd copies).
- ISTFT overlap-add + P/cov scaling via selector matmuls (P/2 baked into the
  P1/P2 stationaries host-side + 2-column edge-correction matmuls).
- bank-grouped R/U layouts so evict/ghost/roll ranges are disjoint (the
  dependency tracker uses bounding boxes; interleaved layouts serialize).
- prologue: frames+DFT in one "crit" DMA alone on the sync queue (queue
  stripes interleave across requests, so big tensors delay small critical
  ones); smat per-bank in fp8 (exact for 0/1) alternating queues.
- software-pipelined issue order so no engine queue is head-of-line blocked.
"""

import numpy as np
from ml_dtypes import bfloat16, float8_e4m3

import concourse.bass as bass
import concourse.bacc as bacc
import concourse.mybir as mybir
import concourse.tile as tile

F = 80
T = 51
TP = 52          # per-j slot stride (51 data + 1 pad/ghost)
HOP = 40
L = 2080
NJ = 20
NCH = 2
CHJ = NJ // NCH  # 10
PBK = 5          # j's per R/U psum bank
BL = CHJ * TP    # 520
FP32 = mybir.dt.float32
BF16 = mybir.dt.bfloat16
FP8 = mybir.dt.float8e4

N2_LISTS = [list(range(19, -1, -1)), list(range(-1, -21, -1))]


def _dft_consts():
    j = np.arange(F)
    W = np.exp(-2j * np.pi * np.outer(j, j) / F)
    G = np.exp(+2j * np.pi * np.outer(j, j) / F) / F
    return W, G


def build_program(debug=False):
    nc = bacc.Bacc("TRN2", target_bir_lowering=False, debug=debug)

    # crit = [xf frames (3T) | fr_c (2F)]: one DMA gates the STFT
    crit = nc.dram_tensor("crit", [F, 3 * T + 2 * F], BF16, kind="ExternalInput")
    # gr_c = [Gr | Gi | GiN | P1 | P2]  (GiN = -Gi; P1/P2 = overlap-add selectors)
    gr_c = nc.dram_tensor("gr_c", [F, 3 * F + 2 * HOP], BF16, kind="ExternalInput")
    smat = nc.dram_tensor("smat", [F, NJ * F], FP8, kind="ExternalInput")
    mst = nc.dram_tensor("mst", [F, NJ * 2 * F], BF16, kind="ExternalInput")
    yv = nc.dram_tensor("yv", [HOP, 2 * 52], FP32, kind="ExternalOutput")

    MUL = mybir.AluOpType.mult
    ADD = mybir.AluOpType.add
    SUB = mybir.AluOpType.subtract
    CPY = mybir.ActivationFunctionType.Copy

    with tile.TileContext(nc) as tc:
        with (
            tc.tile_pool(name="const", bufs=1) as cpool,
            tc.tile_pool(name="work", bufs=1) as wpool,
            tc.tile_pool(name="ps_x", bufs=1, space="PSUM") as ps_x,
            tc.tile_pool(name="ps_r", bufs=2, space="PSUM") as ps_r,
            tc.tile_pool(name="ps_u", bufs=2, space="PSUM") as ps_u,
            tc.tile_pool(name="ps_d", bufs=1, space="PSUM") as ps_d,
            tc.tile_pool(name="ps_y", bufs=1, space="PSUM") as ps_y,
        ):
            # ---- input DMAs; sync queue carries ONLY the critical tensor
            Crit = wpool.tile([F, 3 * T + 2 * F], BF16, tag="Crit")
            nc.sync.dma_start(Crit[:, :], crit[:, :])
            FCO = 3 * T   # Fc column offset within Crit
            Ssb = cpool.tile([F, NJ * F], FP8, tag="Ssb")
            # R stationaries: per-bank (64KB), alternating queues so each bank
            # lands as early as possible regardless of queue jitter
            for q in range(4):
                eng = nc.scalar if q % 2 == 0 else nc.sync
                eng.dma_start(Ssb[:, q * PBK * F:(q + 1) * PBK * F],
                              smat[:, q * PBK * F:(q + 1) * PBK * F])
            Msb = cpool.tile([F, NJ * 2 * F], BF16, tag="Msb")
            nc.gpsimd.dma_start(Msb[:, 0:CHJ * 2 * F], mst[:, 0:CHJ * 2 * F])
            nc.gpsimd.dma_start(Msb[:, CHJ * 2 * F:], mst[:, CHJ * 2 * F:])
            Gc = cpool.tile([F, 3 * F + 2 * HOP], BF16, tag="Gc")
            nc.gpsimd.dma_start(Gc[:, :], gr_c[:, :])

            # D eviction buffer with zero guard columns: per c2 block of 53:
            # [z | t0..t50 | z];  memset once, eviction fills the middle.
            Dsb = wpool.tile([F, 2 * 53], BF16, tag="Dsb")
            nc.gpsimd.memset(Dsb[:, :], 0.0)
            # zero rhs for the PSUM-accumulation opener matmul
            Zsb = wpool.tile([F, 2 * T], BF16, tag="Zsb")
            nc.gpsimd.memset(Zsb[:, :], 0.0)

            # ---- STFT (fp32 accum) -> X bf16 [Xr(52) | Xi(52)] ----
            Xp = ps_x.tile([F, 2 * T], FP32, tag="Xp")
            nc.tensor.matmul(Xp[:, :], Crit[:, FCO:FCO + F], Crit[:, T:3 * T],
                             start=True, stop=False)
            nc.tensor.matmul(Xp[:, :], Crit[:, FCO + F:FCO + 2 * F], Crit[:, 0:2 * T],
                             start=False, stop=True)
            Xsb = wpool.tile([F, 2 * TP], BF16, tag="Xsb")
            Xsv = Xsb[:, :].rearrange("p (c t) -> p c t", c=2)
            nc.scalar.activation(Xsv[:, :, 0:T],
                                 Xp[:, :].rearrange("p (c t) -> p c t", c=2), CPY)
            Xrhs = bass.AP(tensor=Xsb[:, :].tensor, offset=Xsb[:, :].offset,
                           ap=[[2 * TP, F], [TP, 2], [1, T]])
            # X tiled over j (flat TT operands for the C stage), one ACTIVATE
            XtB = wpool.tile([F, 2 * BL], BF16, tag="XtB")
            xsrc = bass.AP(tensor=Xsb[:, :].tensor, offset=Xsb[:, :].offset,
                           ap=[[2 * TP, F], [TP, 2], [0, CHJ], [1, TP]])
            nc.scalar.activation(
                XtB[:, :].rearrange("p (c j t) -> p c j t", c=2, j=CHJ),
                xsrc, CPY)
            Xtr = XtB[:, 0:BL]
            Xti = XtB[:, BL:2 * BL]

            # ---- per-chunk tiles ----
            Rsb, Csb, Usb, Vsb = [], [], [], []
            for c in range(NCH):
                Rsb.append(wpool.tile([F, 2 * BL], BF16, tag=f"Rsb{c}", name=f"Rsb{c}"))
                Csb.append(wpool.tile([F, 3 * BL], BF16, tag=f"Csb{c}", name=f"Csb{c}"))
                Usb.append(wpool.tile([F, 2 * BL], BF16, tag=f"Usb{c}", name=f"Usb{c}"))
                Vsb.append(wpool.tile([F, 2 * BL], BF16, tag=f"Vsb{c}", name=f"Vsb{c}"))
            # R/U layout is bank-grouped: [bank: [r(5*TP) | i(5*TP)]] so each
            # bank's evict/ghost/roll ranges are disjoint (no false deps).
            # Ue has one leading ghost slot per bank: bank*(1+2*HB)+1+...
            HB = PBK * TP   # 260, per-bank component half
            Ue = [wpool.tile([F, 2 * (1 + 2 * HB)], BF16, tag=f"Ue{c}", name=f"Ue{c}")
                  for c in range(NCH)]
            sA = wpool.tile([F, BL], BF16, tag="sA")
            sB = wpool.tile([F, BL], BF16, tag="sB")
            sC = wpool.tile([F, BL], BF16, tag="sC")
            sD = wpool.tile([F, BL], BF16, tag="sD")
            sA2 = wpool.tile([F, BL], BF16, tag="sA2")
            sB2 = wpool.tile([F, BL], BF16, tag="sB2")
            sC2 = wpool.tile([F, BL], BF16, tag="sC2")
            sD2 = wpool.tile([F, BL], BF16, tag="sD2")

            TT = nc.vector.tensor_tensor

            def r_stage(c):
                """R_j = roll(X, n2_j): permutation matmuls, scalar evict."""
                Rc = Rsb[c]
                for bk in range(CHJ // PBK):
                    Rp = ps_r.tile([F, PBK * 2 * T], FP32, tag="Rp")
                    for s in range(PBK):
                        j = c * CHJ + bk * PBK + s
                        nc.tensor.matmul(Rp[:, s * 2 * T:(s + 1) * 2 * T],
                                         Ssb[:, j * F:(j + 1) * F],
                                         Xrhs, start=True, stop=True)
                    dst = bass.AP(tensor=Rc[:, :].tensor,
                                  offset=Rc[:, :].offset + bk * 2 * HB,
                                  ap=[[2 * BL, F], [TP, PBK], [HB, 2], [1, T]])
                    nc.scalar.activation(
                        dst, Rp[:, :].rearrange("p (s c2 t) -> p s c2 t", s=PBK, c2=2),
                        CPY)

            def c_stage(c):
                """C_pre = X * conj(R) -> Csb blocks [CiN | Cr | Ci]."""
                Rc, Cc = Rsb[c], Csb[c]
                Rrf = bass.AP(tensor=Rc[:, :].tensor, offset=Rc[:, :].offset,
                              ap=[[2 * BL, F], [2 * HB, 2], [1, HB]])
                Rif = bass.AP(tensor=Rc[:, :].tensor, offset=Rc[:, :].offset + HB,
                              ap=[[2 * BL, F], [2 * HB, 2], [1, HB]])
                a, b_, c_, d_ = (sA, sB, sC, sD) if c == 0 else (sA2, sB2, sC2, sD2)
                # imag chain first so the scalar CiN negate overlaps the real
                # chain and the U matmuls start right after the last TT
                TT(c_[:, :], Xti[:, :], Rrf, MUL)
                TT(d_[:, :], Xtr[:, :], Rif, MUL)
                TT(Cc[:, 2 * BL:3 * BL], c_[:, :], d_[:, :], SUB)
                nc.scalar.activation(Cc[:, 0:BL], Cc[:, 2 * BL:3 * BL], CPY, scale=-1.0)
                TT(a[:, :], Xtr[:, :], Rrf, MUL)
                TT(b_[:, :], Xti[:, :], Rif, MUL)
                TT(Cc[:, BL:2 * BL], a[:, :], b_[:, :], ADD)

            def u_mm(c):
                """Up_j = Mr@[Cr|Ci] + Mi@[CiN|Cr]; scalar evict; ghost fill."""
                Cc = Csb[c]
                for bk in range(CHJ // PBK):
                    Up = ps_u.tile([F, PBK * 2 * T], FP32, tag="Up")
                    for s in range(PBK):
                        jj = bk * PBK + s
                        j = c * CHJ + jj
                        rhs1 = bass.AP(tensor=Cc[:, :].tensor,
                                       offset=Cc[:, :].offset + BL + jj * TP,
                                       ap=[[3 * BL, F], [BL, 2], [1, T]])
                        rhs2 = bass.AP(tensor=Cc[:, :].tensor,
                                       offset=Cc[:, :].offset + jj * TP,
                                       ap=[[3 * BL, F], [BL, 2], [1, T]])
                        nc.tensor.matmul(Up[:, s * 2 * T:(s + 1) * 2 * T],
                                         Msb[:, (2 * j) * F:(2 * j + 1) * F],
                                         rhs1, start=True, stop=False)
                        nc.tensor.matmul(Up[:, s * 2 * T:(s + 1) * 2 * T],
                                         Msb[:, (2 * j + 1) * F:(2 * j + 2) * F],
                                         rhs2, start=False, stop=True)
                    dst = bass.AP(tensor=Ue[c][:, :].tensor,
                                  offset=Ue[c][:, :].offset + bk * (1 + 2 * HB) + 1,
                                  ap=[[2 * (1 + 2 * HB), F], [TP, PBK], [HB, 2], [1, T]])
                    nc.scalar.activation(
                        dst, Up[:, :].rearrange("p (s c2 t) -> p s c2 t", s=PBK, c2=2),
                        CPY)

            def u_ghost(c, bk):
                """Per-bank ghosts (vector): slot before each block <- its t=50."""
                ob = bk * (1 + 2 * HB)
                gdst = bass.AP(tensor=Ue[c][:, :].tensor,
                               offset=Ue[c][:, :].offset + ob,
                               ap=[[2 * (1 + 2 * HB), F], [HB, 2], [TP, PBK], [1, 1]])
                gsrc = bass.AP(tensor=Ue[c][:, :].tensor,
                               offset=Ue[c][:, :].offset + ob + T,
                               ap=[[2 * (1 + 2 * HB), F], [HB, 2], [TP, PBK], [1, 1]])
                nc.gpsimd.tensor_copy(gdst, gsrc)

            def u_roll(c, bk):
                """Per-bank flat TT: U = Ue[1:] + Ue[:-1] (ghosts wrap t=0)."""
                dst = bass.AP(tensor=Usb[c][:, :].tensor,
                              offset=Usb[c][:, :].offset + bk * 2 * HB,
                              ap=[[2 * BL, F], [1, 2 * HB]])
                s1 = bass.AP(tensor=Ue[c][:, :].tensor,
                             offset=Ue[c][:, :].offset + bk * (1 + 2 * HB) + 1,
                             ap=[[2 * (1 + 2 * HB), F], [1, 2 * HB]])
                s0 = bass.AP(tensor=Ue[c][:, :].tensor,
                             offset=Ue[c][:, :].offset + bk * (1 + 2 * HB),
                             ap=[[2 * (1 + 2 * HB), F], [1, 2 * HB]])
                TT(dst, s1, s0, ADD)

            def v_stage(c):
                """V = U * R -> Vsb blocks [Vr | Vi] (no ViN: -Gi stationary)."""
                Rc, Uc, Vc = Rsb[c], Usb[c], Vsb[c]

                def half(tile_, o):
                    return bass.AP(tensor=tile_[:, :].tensor,
                                   offset=tile_[:, :].offset + o,
                                   ap=[[2 * BL, F], [2 * HB, 2], [1, HB]])
                Rrf, Rif = half(Rc, 0), half(Rc, HB)
                Urf, Uif = half(Uc, 0), half(Uc, HB)
                a, b_, c_, d_ = (sA, sB, sC, sD) if c == 0 else (sA2, sB2, sC2, sD2)
                # Vr chain first: the Gi@Vr / Gr@Vr matmuls start while the
                # Vi chain still runs on vector
                TT(a[:, :], Urf, Rrf, MUL)
                TT(b_[:, :], Uif, Rif, MUL)
                TT(Vc[:, 0:BL], a[:, :], b_[:, :], SUB)
                TT(c_[:, :], Urf, Rif, MUL)
                TT(d_[:, :], Uif, Rrf, MUL)
                TT(Vc[:, BL:2 * BL], c_[:, :], d_[:, :], ADD)

            Dp = ps_d.tile([F, 2 * T], FP32, tag="Dp")

            def g_stage(c, start, stop):
                """D += sum_j G @ V_j : zero-stride dst accumulates j in PSUM.

                Dr += Gr@Vr - Gi@Vi ; Di += Gr@Vi + Gi@Vr.  gpass0 does Gr on
                the [Vr|Vi] pair; gpass1 uses GiN=-Gi on Vi (Dr) and Gi on Vr
                (Di), so no negated V copy is ever materialized.
                """
                Vc = Vsb[c]
                dstR = bass.AP(tensor=Dp[:, :].tensor, offset=Dp[:, :].offset,
                               ap=[[2 * T, F], [0, PBK], [1, T]])
                dstI = bass.AP(tensor=Dp[:, :].tensor, offset=Dp[:, :].offset + T,
                               ap=[[2 * T, F], [0, PBK], [1, T]])

                def vr(h):
                    return bass.AP(tensor=Vc[:, :].tensor,
                                   offset=Vc[:, :].offset + h * PBK * TP,
                                   ap=[[2 * BL, F], [TP, PBK], [1, T]])

                def vi(h):
                    return bass.AP(tensor=Vc[:, :].tensor,
                                   offset=Vc[:, :].offset + BL + h * PBK * TP,
                                   ap=[[2 * BL, F], [TP, PBK], [1, T]])

                # only ONE accumulation window may be open per PSUM bank, so a
                # zero-matmul opens (and zeroes) the full region; every real
                # mm accumulates; the last one closes the window.
                if start:
                    dfull = bass.AP(tensor=Dp[:, :].tensor, offset=Dp[:, :].offset,
                                    ap=[[2 * T, F], [1, 2 * T]])
                    nc.tensor.matmul(dfull, Gc[:, 0:F], Zsb[:, :],
                                     start=True, stop=False,
                                     skip_group_check=True)
                for h in range(CHJ // PBK):
                    nc.tensor.matmul(dstI, Gc[:, F:2 * F], vr(h),
                                     start=False, stop=False,
                                     skip_group_check=True)
                for h in range(CHJ // PBK):
                    nc.tensor.matmul(dstR, Gc[:, 0:F], vr(h),
                                     start=False, stop=False,
                                     skip_group_check=True)
                for h in range(CHJ // PBK):
                    nc.tensor.matmul(dstI, Gc[:, 0:F], vi(h),
                                     start=False, stop=False,
                                     skip_group_check=True)
                for h in range(CHJ // PBK):
                    nc.tensor.matmul(dstR, Gc[:, 2 * F:3 * F], vi(h),
                                     start=False,
                                     stop=(stop and h == CHJ // PBK - 1),
                                     skip_group_check=True)

            # ---------- pipelined issue order ----------
            # vector queue: C0(6), C1(6), r0a, r0b, V0(6), r1a, r1b, V1(6), Y
            r_stage(0)
            r_stage(1)
            c_stage(0)
            u_mm(0)
            c_stage(1)
            u_ghost(0, 0)
            u_roll(0, 0)
            u_ghost(0, 1)
            u_roll(0, 1)
            v_stage(0)
            u_mm(1)
            g_stage(0, start=True, stop=False)
            u_ghost(1, 0)
            u_roll(1, 0)
            u_ghost(1, 1)
            u_roll(1, 1)
            v_stage(1)
            g_stage(1, start=False, stop=True)

            # ---------- tail: evict D, overlap-add via selector matmuls ----------
            dce = bass.AP(tensor=Dsb[:, :].tensor, offset=Dsb[:, :].offset + 1,
                          ap=[[2 * 53, F], [53, 2], [1, T]])
            nc.scalar.activation(dce, Dp[:, :].rearrange("p (c t) -> p c t", c=2), CPY)
            Yp = ps_y.tile([HOP, 2 * 52], FP32, tag="Yp")
            # y[tau,c2,tp] = (P/2)*(D[tau,c2,tp] + D[tau+40,c2,tp-1]); the P/2
            # scale is baked into the selector stationaries host-side, and the
            # edge samples (tp=0, tp=51, where cov=1 not 2) get one extra
            # accumulation each via 2-column correction matmuls.
            dstY = bass.AP(tensor=Yp[:, :].tensor, offset=Yp[:, :].offset,
                           ap=[[2 * 52, HOP], [52, 2], [1, 52]])
            rhs1 = bass.AP(tensor=Dsb[:, :].tensor, offset=Dsb[:, :].offset + 1,
                           ap=[[2 * 53, F], [53, 2], [1, 52]])
            rhs2 = bass.AP(tensor=Dsb[:, :].tensor, offset=Dsb[:, :].offset,
                           ap=[[2 * 53, F], [53, 2], [1, 52]])
            nc.tensor.matmul(dstY, Gc[:, 3 * F:3 * F + HOP], rhs1,
                             start=True, stop=False)
            nc.tensor.matmul(dstY, Gc[:, 3 * F + HOP:3 * F + 2 * HOP], rhs2,
                             start=False, stop=False)
            dstY0 = bass.AP(tensor=Yp[:, :].tensor, offset=Yp[:, :].offset,
                            ap=[[2 * 52, HOP], [52, 2], [1, 1]])
            rhsD0 = bass.AP(tensor=Dsb[:, :].tensor, offset=Dsb[:, :].offset + 1,
                            ap=[[2 * 53, F], [53, 2], [1, 1]])
            nc.tensor.matmul(dstY0, Gc[:, 3 * F:3 * F + HOP], rhsD0,
                             start=False, stop=False, skip_group_check=True)
            dstY1 = bass.AP(tensor=Yp[:, :].tensor, offset=Yp[:, :].offset + T,
                            ap=[[2 * 52, HOP], [52, 2], [1, 1]])
            rhsD1 = bass.AP(tensor=Dsb[:, :].tensor, offset=Dsb[:, :].offset + T,
                            ap=[[2 * 53, F], [53, 2], [1, 1]])
            nc.tensor.matmul(dstY1, Gc[:, 3 * F + HOP:3 * F + 2 * HOP], rhsD1,
                             start=False, stop=True, skip_group_check=True)
            Y = wpool.tile([HOP, 2 * 52], FP32, tag="Y")
            nc.scalar.activation(Y[:, :], Yp[:, :], CPY)
            nc.sync.dma_start(yv[:, :], Y[:, :])
    return nc


# ---------------- host side ----------------

def _host_consts():
    W, G = _dft_consts()
    fr_c = np.concatenate([W.real, W.imag], axis=1).astype(bfloat16)
    P1 = np.zeros((F, HOP), np.float32)
    P2 = np.zeros((F, HOP), np.float32)
    P1[np.arange(HOP), np.arange(HOP)] = 1.0
    P2[HOP + np.arange(HOP), np.arange(HOP)] = 1.0
    cov = np.zeros(L)
    idx = (np.arange(T)[:, None] * HOP + np.arange(F)[None, :]).reshape(-1)
    np.add.at(cov, idx, 1.0)
    cov = np.where(cov > 0, cov, 1.0)
    return fr_c, (G, P1, P2), cov


def _smat_for(n2_list):
    S = np.zeros((NJ, F, F), np.float32)
    g = np.arange(F)
    for j, n2 in enumerate(n2_list):
        S[j, (g - n2) % F, g] = 1.0
    return np.ascontiguousarray(
        S.transpose(1, 0, 2).reshape(F, NJ * F)).astype(float8_e4m3)


def _mst_for(n2_list, w2):
    Ms = np.zeros((NJ, 2, F, F), np.float32)
    g = np.arange(F)[:, None]
    f = np.arange(F)[None, :]
    n1 = ((f - g + 20) % F) - 20
    valid = (n1 >= -20) & (n1 <= 19)
    n1c = np.clip(n1 + 20, 0, 39)
    for j, n2 in enumerate(n2_list):
        col = w2[:, n2 + 20]
        Ms[j, 0] = np.where(valid, col.real[n1c], 0.0)
        Ms[j, 1] = np.where(valid, col.imag[n1c], 0.0)
    return np.ascontiguousarray(
        Ms.transpose(2, 0, 1, 3).reshape(F, NJ * 2 * F)).astype(bfloat16)


def _frame(sig):
    idx = np.arange(T)[None, :] * HOP + np.arange(F)[:, None]   # [j, t]
    return sig[idx].astype(np.float32)


def make_in_maps(x_real, x_imag, task_info, w_real, w_imag):
    fr_c, (G, P1, P2), cov = _host_consts()
    b, _, m = x_real.shape
    P = np.power(10.0, task_info[:, 0] / 10.0) / m
    w2 = (np.asarray(w_real) + 1j * np.asarray(w_imag)).reshape(40, 40)
    smats = [_smat_for(nl) for nl in N2_LISTS]
    msts = [_mst_for(nl, w2) for nl in N2_LISTS]
    gr_cs = [np.concatenate(
        [G.real, G.imag, -G.imag, (P[bb] / 2) * P1, (P[bb] / 2) * P2],
        axis=1).astype(bfloat16) for bb in range(b)]

    in_maps, shards = [], []
    for bb in range(b):
        for mm in range(m):
            fr_ = _frame(x_real[bb, :, mm])
            fi_ = _frame(x_imag[bb, :, mm])
            critv = np.concatenate(
                [np.concatenate([-fi_, fr_, fi_], axis=1).astype(bfloat16), fr_c],
                axis=1)
            for h in range(2):
                in_maps.append({
                    "crit": critv,
                    "gr_c": gr_cs[bb],
                    "smat": smats[h],
                    "mst": msts[h],
                })
                shards.append((bb, mm, h))
    return in_maps, shards, P, cov


_NC_CACHE = {}


def kernel(x_real, x_imag, task_info, w_real, w_imag, b_real, b_imag):
    x_real = np.asarray(x_real)
    x_imag = np.asarray(x_imag)
    task_info = np.asarray(task_info)
    b, Lx, m = x_real.shape
    assert (b, Lx, m) == (2, L, 2)

    if "nc" not in _NC_CACHE:
        nc_ = build_program(debug=False)
        nc_.compile()
        _NC_CACHE["nc"] = nc_
    nc = _NC_CACHE["nc"]

    in_maps, shards, P, cov = make_in_maps(x_real, x_imag, task_info, w_real, w_imag)
    from concourse.bass_utils import run_bass_kernel_spmd
    res = run_bass_kernel_spmd(nc, in_maps, list(range(8))).results

    x = (x_real + 1j * x_imag).astype(np.complex64)
    out = x.copy()
    bias = complex(np.asarray(b_real)[0], np.asarray(b_imag)[0])
    bias_sig = np.zeros(L, np.complex64)
    bias_sig[np.arange(T) * HOP] = bias
    bias_sig /= cov
    for i, (bb, mm, h) in enumerate(shards):
        yvv = res[i]["yv"]          # [40, 104] = [tau, (yr(52) | yi(52))]
        yr = yvv[:, 0:52].T.ravel()[:L]
        yi = yvv[:, 52:104].T.ravel()[:L]
        out[bb, :, mm] += yr + 1j * yi
    for bb in range(b):
        for mm in range(m):
            out[bb, :, mm] += (P[bb] * bias_sig).astype(np.complex64)
    return out[:, 20:L - 20, :]
